# revision 2
# baseline (speedup 1.0000x reference)
"""Trainium2 Bass kernel for nn_GroupedVisitEncoder.

Reference computes, per token t (of B*T=65536), from x[t, 0:64]:
  gv[t,g]   = segment-mean of x over 16 contiguous groups (sizes 1,3,5,7 cycling)
  tokens    = gv[...,None]*w_gp + b_gp                  [B,T,16,256]  (1 GiB)
  scores    = tanh(tokens) @ w_sp + b_sp                [B,T,16]
  weights   = softmax(scores, axis=-1)                  [B,T,16]
  visit     = einsum('btg,btgh->bth', weights, tokens)  [B,T,256]

Key algebraic facts exploited here:
  * tokens = x_aug @ W2_aug where x_aug=[x,1] and W2_aug[f,(g,h)]=S[f,g]*w_gp[h]
    (S = segment-mean matrix), bias row = tile(b_gp,16). One PE matmul.
  * scores = f(gv) pointwise, f(v) = sum_h w_sp[h]*tanh(v*w_gp[h]+b_gp[h]).
    f is evaluated as a degree-24 polynomial fitted host-side per call
    (coefficients are runtime *input tensors*, so compilation stays
    value-independent). b_sp drops out (softmax shift invariance).
  * visit = s1*w_gp + b_gp with s1 = sum_g weights*gv (softmax weights sum to 1).

Sharding: pure data parallel, 8192 tokens per core on 8 cores.
"""

import os
import sys
import numpy as np

for _p in ("/opt/trn_rl_repo", os.path.expanduser("~/.axon_site/_ro/trn_rl_repo")):
    if os.path.isdir(_p) and _p not in sys.path:
        sys.path.insert(0, _p)

import concourse.bass as bass
import concourse.bacc as bacc
import concourse.tile as tile
from concourse import mybir
from concourse.bass_utils import run_bass_kernel_spmd

F32 = mybir.dt.float32
N_CORES = 8
B, T, F, G, H = 32, 2048, 64, 16, 256
NTOK = B * T // N_CORES            # 8192 tokens per core
P = 128                            # partitions per tile
NTILES = NTOK // P                 # 64 tiles
TILES_PER_CHUNK = 16
NCHUNK = NTILES // TILES_PER_CHUNK # 4
GH = G * H                         # 4096
DEG = 24                           # polynomial degree for f(gv)

GROUP_SIZES = np.tile(np.array([1, 3, 5, 7]), 4)
SEG_IDS = np.repeat(np.arange(16), GROUP_SIZES)

_COMPILED = {}


def _build_bass():
    nc = bacc.Bacc()
    xs = nc.declare_dram_parameter("xs", [NTOK, F], F32, isOutput=False)
    W2a = nc.declare_dram_parameter("W2a", [F + 1, GH], F32, isOutput=False)
    Sa = nc.declare_dram_parameter("Sa", [F + 1, G], F32, isOutput=False)
    wgpb = nc.declare_dram_parameter("wgpb", [P, H], F32, isOutput=False)
    bgpb = nc.declare_dram_parameter("bgpb", [P, H], F32, isOutput=False)
    coefs = nc.declare_dram_parameter("coefs", [P, DEG + 1], F32, isOutput=False)
    invv = nc.declare_dram_parameter("invv", [P, 1], F32, isOutput=False)
    ident = nc.declare_dram_parameter("ident", [P, P], F32, isOutput=False)
    tok_o = nc.declare_dram_parameter("tok_o", [NTOK, GH], F32, isOutput=True)
    vis_o = nc.declare_dram_parameter("vis_o", [NTOK, H], F32, isOutput=True)
    wts_o = nc.declare_dram_parameter("wts_o", [NTOK, G], F32, isOutput=True)

    wts_view = wts_o.rearrange("(i p) g -> p i g", p=P)  # [128, 64, 16]

    with tile.TileContext(nc) as tc:
        with tc.tile_pool(name="consts", bufs=1) as consts:
            W2_sb = consts.tile([F + 1, GH], F32)
            nc.sync.dma_start(out=W2_sb, in_=W2a[:, :])
            Sa_sb = consts.tile([F + 1, G], F32)
            nc.sync.dma_start(out=Sa_sb, in_=Sa[:, :])
            wgp_sb = consts.tile([P, H], F32)
            nc.sync.dma_start(out=wgp_sb, in_=wgpb[:, :])
            bgp_sb = consts.tile([P, H], F32)
            nc.sync.dma_start(out=bgp_sb, in_=bgpb[:, :])
            co_sb = consts.tile([P, DEG + 1], F32)
            nc.sync.dma_start(out=co_sb, in_=coefs[:, :])
            iv_sb = consts.tile([P, 1], F32)
            nc.sync.dma_start(out=iv_sb, in_=invv[:, :])
            id_sb = consts.tile([P, P], F32)
            nc.sync.dma_start(out=id_sb, in_=ident[:, :])
            xT = consts.tile([F + 1, NTOK], F32)  # transposed x + ones row
            nc.vector.memset(xT[F : F + 1, :], 1.0)

            # Phase 0: transpose x into [f, tok] layout via the PE.
            with (
                tc.tile_pool(name="xload", bufs=4) as xload,
                tc.tile_pool(name="tp_psum", bufs=4, space="PSUM") as tp_psum,
            ):
                for i in range(NTILES):
                    x_nat = xload.tile([P, F], F32)
                    nc.sync.dma_start(out=x_nat, in_=xs[i * P : (i + 1) * P, :])
                    ps_tp = tp_psum.tile([F, P], F32)
                    nc.tensor.transpose(ps_tp, x_nat, id_sb)
                    if i % 2 == 0:
                        nc.scalar.copy(out=xT[0:F, i * P : (i + 1) * P], in_=ps_tp)
                    else:
                        nc.vector.tensor_copy(out=xT[0:F, i * P : (i + 1) * P], in_=ps_tp)

            with (
                tc.tile_pool(name="tok_psum", bufs=3, space="PSUM") as tok_psum,
                tc.tile_pool(name="gv_psum", bufs=2, space="PSUM") as gv_psum,
                tc.tile_pool(name="tok_pool", bufs=3) as tok_pool,
                tc.tile_pool(name="gv_pool", bufs=2) as gv_pool,
                tc.tile_pool(name="hp", bufs=4) as hp,
                tc.tile_pool(name="vp", bufs=4) as vp,
            ):
                for c in range(NCHUNK):
                    gv_ch = gv_pool.tile([P, TILES_PER_CHUNK * G], F32)  # [128, 256]
                    for t in range(TILES_PER_CHUNK):
                        i = c * TILES_PER_CHUNK + t
                        lhsT = xT[:, i * P : (i + 1) * P]
                        ps_gv = gv_psum.tile([P, G], F32)
                        nc.tensor.matmul(ps_gv, lhsT, Sa_sb[:, :], start=True, stop=True)
                        nc.scalar.copy(out=gv_ch[:, t * G : (t + 1) * G], in_=ps_gv)
                        tok_sb = tok_pool.tile([P, GH], F32)
                        for j in range(4):
                            ps = tok_psum.tile([P, 1024], F32)
                            nc.tensor.matmul(
                                ps[:, 0:512], lhsT,
                                W2_sb[:, j * 1024 : j * 1024 + 512],
                                start=True, stop=True,
                            )
                            nc.tensor.matmul(
                                ps[:, 512:1024], lhsT,
                                W2_sb[:, j * 1024 + 512 : (j + 1) * 1024],
                                start=True, stop=True,
                            )
                            dst = tok_sb[:, j * 1024 : (j + 1) * 1024]
                            if j % 2 == 0:
                                nc.scalar.copy(out=dst, in_=ps)
                            else:
                                nc.vector.tensor_copy(out=dst, in_=ps)
                        nc.sync.dma_start(
                            out=tok_o[i * P : (i + 1) * P, :], in_=tok_sb
                        )

                    # --- score/visit pipeline for this chunk of 16 tiles ---
                    NW = TILES_PER_CHUNK * G  # 256
                    u = hp.tile([P, NW], F32)
                    nc.vector.tensor_scalar(
                        out=u, in0=gv_ch, scalar1=iv_sb[:, 0:1], scalar2=None,
                        op0=mybir.AluOpType.mult,
                    )
                    p_h = hp.tile([P, NW], F32)
                    nc.vector.tensor_scalar(
                        out=p_h, in0=u, scalar1=co_sb[:, 0:1], scalar2=co_sb[:, 1:2],
                        op0=mybir.AluOpType.mult, op1=mybir.AluOpType.add,
                    )
                    for k in range(2, DEG + 1):
                        tmp = hp.tile([P, NW], F32)
                        nc.vector.tensor_tensor(tmp, p_h, u, mybir.AluOpType.mult)
                        p_h = hp.tile([P, NW], F32)
                        nc.vector.tensor_scalar(
                            out=p_h, in0=tmp, scalar1=co_sb[:, k : k + 1],
                            scalar2=None, op0=mybir.AluOpType.add,
                        )
                    e = hp.tile([P, TILES_PER_CHUNK, G], F32)
                    nc.scalar.activation(
                        out=e, in_=p_h.rearrange("p (t g) -> p t g", g=G),
                        func=mybir.ActivationFunctionType.Exp,
                    )
                    se = hp.tile([P, TILES_PER_CHUNK], F32)
                    nc.vector.tensor_reduce(
                        out=se, in_=e, axis=mybir.AxisListType.X, op=mybir.AluOpType.add
                    )
                    r = hp.tile([P, TILES_PER_CHUNK], F32)
                    nc.vector.reciprocal(out=r, in_=se)
                    w = hp.tile([P, TILES_PER_CHUNK, G], F32)
                    nc.vector.tensor_tensor(
                        w, e,
                        r[:, :, None].to_broadcast((P, TILES_PER_CHUNK, G)),
                        mybir.AluOpType.mult,
                    )
                    nc.gpsimd.dma_start(
                        out=wts_view[:, c * TILES_PER_CHUNK : (c + 1) * TILES_PER_CHUNK, :],
                        in_=w,
                    )
                    wgv = hp.tile([P, TILES_PER_CHUNK, G], F32)
                    nc.vector.tensor_tensor(
                        wgv, w, gv_ch.rearrange("p (t g) -> p t g", g=G),
                        mybir.AluOpType.mult,
                    )
                    s1 = hp.tile([P, TILES_PER_CHUNK], F32)
                    nc.vector.tensor_reduce(
                        out=s1, in_=wgv, axis=mybir.AxisListType.X, op=mybir.AluOpType.add
                    )
                    for t in range(TILES_PER_CHUNK):
                        i = c * TILES_PER_CHUNK + t
                        vs = vp.tile([P, H], F32)
                        nc.vector.tensor_scalar(
                            out=vs, in0=wgp_sb, scalar1=s1[:, t : t + 1],
                            scalar2=None, op0=mybir.AluOpType.mult,
                        )
                        vs2 = vp.tile([P, H], F32)
                        nc.gpsimd.tensor_tensor(vs2, vs, bgp_sb, mybir.AluOpType.add)
                        nc.gpsimd.dma_start(
                            out=vis_o[i * P : (i + 1) * P, :], in_=vs2
                        )
    nc.finalize()
    return nc


def _get_nc():
    if "nc" not in _COMPILED:
        _COMPILED["nc"] = _build_bass()
    return _COMPILED["nc"]


def _host_prep(x, w_gp, b_gp, w_sp):
    x = np.asarray(x, np.float32)
    w_gp = np.asarray(w_gp, np.float32)
    b_gp = np.asarray(b_gp, np.float32)
    w_sp = np.asarray(w_sp, np.float32)

    S = np.zeros((F, G), np.float64)
    S[np.arange(F), SEG_IDS] = 1.0 / GROUP_SIZES[SEG_IDS]
    W2 = (S[:, :, None] * w_gp.astype(np.float64)[None, None, :]).reshape(F, GH)
    W2a = np.vstack([W2, np.tile(b_gp, G)[None, :]]).astype(np.float32)
    Sa = np.vstack([S, np.zeros((1, G))]).astype(np.float32)

    vmax = float(np.abs(x).max()) * 1.02
    grid = np.linspace(-1.0, 1.0, 8001)
    Ftrue = np.tanh((grid[:, None] * vmax) * w_gp.astype(np.float64)
                    + b_gp.astype(np.float64)) @ w_sp.astype(np.float64)
    coefs = np.polyfit(grid, Ftrue, DEG)
    fit_err = float(np.abs(np.polyval(coefs, grid) - Ftrue).max())

    rep = lambda a, n: np.broadcast_to(np.asarray(a, np.float32), (P, n)).copy()
    return {
        "W2a": W2a,
        "Sa": Sa,
        "wgpb": rep(w_gp, H),
        "bgpb": rep(b_gp, H),
        "coefs": rep(coefs.astype(np.float32), DEG + 1),
        "invv": np.full((P, 1), 1.0 / vmax, np.float32),
        "ident": np.eye(P, dtype=np.float32),
    }, fit_err


def _run(x, w_gp, b_gp, w_sp, trace=False):
    nc = _get_nc()
    shared, fit_err = _host_prep(x, w_gp, b_gp, w_sp)
    x_flat = np.ascontiguousarray(np.asarray(x, np.float32).reshape(B * T, F))
    in_maps = []
    for i in range(N_CORES):
        m = dict(shared)
        m["xs"] = np.ascontiguousarray(x_flat[i * NTOK : (i + 1) * NTOK])
        in_maps.append(m)
    res = run_bass_kernel_spmd(nc, in_maps, list(range(N_CORES)), trace=trace)
    return res, fit_err


def kernel(x, w_gp, b_gp, w_sp, b_sp=None, **_ignored):
    res, _ = _run(x, w_gp, b_gp, w_sp)
    toks = np.concatenate([res.results[i]["tok_o"] for i in range(N_CORES)], axis=0)
    vis = np.concatenate([res.results[i]["vis_o"] for i in range(N_CORES)], axis=0)
    wts = np.concatenate([res.results[i]["wts_o"] for i in range(N_CORES)], axis=0)
    visit = vis.reshape(B, T, H)
    tokens = toks.reshape(B, T, G, H)
    weights = wts.reshape(B, T, G)
    return visit, tokens, weights
